# revision 1
# baseline (speedup 1.0000x reference)
"""Causal self-attention (quirky-reshape variant) on 8 TRN2 NeuronCores.

Key structural fact: the reference reshapes (B,S,H*dk) -> (B,H,S,dk) without a
transpose, so head h's Q/K/V come from rows [256h, 256h+256) of the [4096,1024]
projection output (reinterpreted as [4096,64]), and output rows [256h, 256h+256)
depend only on head h.  With 2 heads per core the problem is embarrassingly
parallel: core i consumes x rows [512i, 512i+512) + full weights and produces
output rows [512i, 512i+512).  No collectives.

Per-core pipeline (all matmuls in the "transposed-scores" orientation):
  qflat^T/kflat^T via o-stationary projection (host-permuted weights fold both
  the quirky reshape's d-extraction and a RoPE even/odd de-interleave into the
  PSUM partition order), vflat via r-stationary projection; RoPE as 3 full-width
  DVE passes; scores^T = K^T-stationary matmul (2 heads row-packed in the PE
  array); exp on ScalarE (scale=1/8 folded in, causal triangle masks added on
  PSUM, fully-masked columns skipped by ragged matmul widths); P@V with a
  [V|ones] stationary (ones column yields softmax denominators); normalize with
  reciprocal_approx_fast + a PE outer-product broadcast; output projection from
  strided-gathered concat^T tiles.
"""

import os

os.environ.setdefault("JAX_PLATFORMS", "cpu")

import numpy as np
import ml_dtypes

D = 1024          # d_model
H = 16            # heads
DK = 64           # head dim
S = 4096          # seq len
NC_N = 8          # cores
RPC = 512         # x rows per core
HPC = 2           # heads per core
NT_SK = 32        # sk tiles of 128 per head
ROPE_THETA = 10000.0
F32 = None        # set at build (mybir.dt.float32)
BF16 = None

_CACHE = {}


def _deint_perm():
    """o' -> o source index: within each 64-block, evens first then odds."""
    d_order = list(range(0, DK, 2)) + list(range(1, DK, 2))  # position d' -> d
    perm = np.zeros(D, dtype=np.int64)
    for c in range(H):
        for dp, d in enumerate(d_order):
            perm[c * DK + dp] = c * DK + d
    return perm


def _host_arrays(Wq, Wk, Wv, Wo):
    bf = ml_dtypes.bfloat16
    perm = _deint_perm()
    wqT = np.ascontiguousarray(Wq[perm, :].T).astype(bf)   # [in, o'] deint
    wkT = np.ascontiguousarray(Wk[perm, :].T).astype(bf)
    wvT = np.ascontiguousarray(Wv.T).astype(bf)            # [in, o] natural
    woT = np.ascontiguousarray(Wo.T).astype(bf)            # [o_c, o_out]

    # RoPE tables in the [Aev, Aod, Bev, Bod] partition grouping (32 rows each;
    # identical for both heads since the angle depends only on s).
    j = np.arange(0, DK, 2, dtype=np.float64) / DK
    inv_freq = 1.0 / (ROPE_THETA ** j)                     # [32]
    # Q storage is c-major: u = 256*c + r <-> s = 16*r + c; K is s-ordered
    u = np.arange(S)
    s_of_u = 16 * (u % 256) + u // 256
    angq = np.outer(inv_freq, s_of_u)                      # [32, S] u-ordered
    angk = np.outer(inv_freq, np.arange(S))                # [32, S] s-ordered
    csq1 = np.concatenate([np.cos(angq)] * 4, 0).astype(bf)
    csq2 = np.concatenate([-np.sin(angq), np.sin(angq)] * 2, 0).astype(bf)
    csk1 = np.concatenate([np.cos(angk)] * 4, 0).astype(bf)
    csk2 = np.concatenate([-np.sin(angk), np.sin(angk)] * 2, 0).astype(bf)
    # triangle mask: row p = local sk (plain), col j = 8*cq + rq (c-major sq)
    p = np.arange(128)
    cq, rq = np.arange(128) // 8, np.arange(128) % 8
    sq_loc = 16 * rq + cq                                  # [128]
    tri = np.where(
        p[:, None] <= sq_loc[None, :], 0.0, -1e30
    ).astype(np.float32)
    return wqT, wkT, wvT, woT, csq1, csq2, csk1, csk2, tri


def _build_program(dbg=False):
    import concourse.bass as bass
    import concourse.tile as tile
    from concourse import bacc, mybir

    f32 = mybir.dt.float32
    bf16 = mybir.dt.bfloat16
    EXP = mybir.ActivationFunctionType.Exp
    CPY = mybir.ActivationFunctionType.Copy

    nc = bacc.Bacc("TRN2", target_bir_lowering=False, debug=False,
                   num_devices=NC_N)

    xT = nc.dram_tensor("xT", [D, RPC], bf16, kind="ExternalInput").ap()
    wq = nc.dram_tensor("wqT", [D, D], bf16, kind="ExternalInput").ap()
    wk = nc.dram_tensor("wkT", [D, D], bf16, kind="ExternalInput").ap()
    wv = nc.dram_tensor("wvT", [D, D], bf16, kind="ExternalInput").ap()
    wo = nc.dram_tensor("woT", [D, D], bf16, kind="ExternalInput").ap()
    cs1d = nc.dram_tensor("cs1", [128, S], bf16, kind="ExternalInput").ap()
    cs2d = nc.dram_tensor("cs2", [128, S], bf16, kind="ExternalInput").ap()
    cs3d = nc.dram_tensor("cs3", [128, S], bf16, kind="ExternalInput").ap()
    cs4d = nc.dram_tensor("cs4", [128, S], bf16, kind="ExternalInput").ap()
    trid = nc.dram_tensor("tri", [128, 128], f32, kind="ExternalInput").ap()
    y = nc.dram_tensor("y", [RPC, D], f32, kind="ExternalOutput").ap()
    vfd = nc.dram_tensor("vflat_scratch", [RPC, D], bf16).ap()
    dbg_aps = {}
    if dbg:
        for nm, shp, dt in [
            ("dbg_qraw", [128, S], bf16), ("dbg_kraw", [128, S], bf16),
            ("dbg_qrot", [128, S], bf16), ("dbg_krot", [128, S], bf16),
            ("dbg_vsb0", [128, 65 * NT_SK], bf16),
            ("dbg_vsb1", [128, 65 * NT_SK], bf16),
            ("dbg_outT0", [64, S], bf16), ("dbg_outT1", [64, S], bf16),
            ("dbg_vflat", [RPC, D], bf16),
            ("dbg_outp0", [65, 512], f32), ("dbg_ps0", [128, 1536], f32),
            ("dbg_pch0", [128, 1536], bf16), ("dbg_dn0", [1, 512], f32),
            ("dbg_bc0", [64, 512], f32),
        ]:
            dbg_aps[nm] = nc.dram_tensor(nm, shp, dt, kind="ExternalOutput").ap()

    with tile.TileContext(nc) as tc:
        with (
            tc.tile_pool(name="big", bufs=3) as big,        # wq/wk/wv -> outTA/outTB/y_sb
            tc.tile_pool(name="wo", bufs=1) as wop,
            tc.tile_pool(name="xp", bufs=1) as xp,
            tc.tile_pool(name="qk", bufs=2) as qkp,          # qraw, kraw (become rot in place)
            tc.tile_pool(name="rope", bufs=5) as ropep,      # cs tables + swap
            tc.tile_pool(name="vf", bufs=1) as vfp,
            tc.tile_pool(name="vsb", bufs=2) as vsbp,
            tc.tile_pool(name="mask", bufs=1) as maskp,
            tc.tile_pool(name="pp", bufs=3) as ppool,        # exp'd P chunks
            tc.tile_pool(name="ct", bufs=2) as ctp,          # concatT per (h, rt)
            tc.tile_pool(name="misc", bufs=1) as miscp,
        ):
            # ---------- phase 0: loads ----------
            xsb = xp.tile([128, 8 * RPC], bf16, tag="x")           # [p, kt*512+r]
            nc.sync.dma_start(xsb[:].rearrange("p (kt r) -> p kt r", kt=8),
                              xT.rearrange("(kt p) r -> p kt r", p=128))

            def load_w(pool, tag, src):
                t = pool.tile([128, 8 * D], bf16, tag=tag, name=f"w_{tag}")
                tv = t[:].rearrange("p (kt o) -> p kt o", kt=8)
                sv = src.rearrange("(kt p) o -> p kt o", p=128)
                for kt in range(8):
                    nc.sync.dma_start(tv[:, kt, :], sv[:, kt, :])
                return t

            wq_sb = load_w(big, "big", wq)
            wk_sb = load_w(big, "big", wk)
            wv_sb = load_w(big, "big", wv)
            wo_sb = load_w(wop, "wo", wo)
            cs1_sb = ropep.tile([128, S], bf16, tag="rope")
            nc.sync.dma_start(cs1_sb[:], cs1d[:])
            cs2_sb = ropep.tile([128, S], bf16, tag="rope")
            nc.sync.dma_start(cs2_sb[:], cs2d[:])
            cs3_sb = ropep.tile([128, S], bf16, tag="rope")
            nc.sync.dma_start(cs3_sb[:], cs3d[:])
            cs4_sb = ropep.tile([128, S], bf16, tag="rope")
            nc.sync.dma_start(cs4_sb[:], cs4d[:])
            tri_sb = maskp.tile([128, 128], f32, tag="mask")
            nc.sync.dma_start(tri_sb[:], trid[:])

            misc = miscp.tile([128, 2048], f32, tag="misc")
            nc.gpsimd.memset(misc[:], 0.0)
            bc_sb = miscp.tile([64, 512], f32, tag="bc")

            # ---------- phase 1: projections ----------
            qraw = qkp.tile([128, S], bf16, tag="qk")   # [Aev,Aod,Bev,Bod] x s
            kraw = qkp.tile([128, S], bf16, tag="qk")

            with tc.tile_pool(name="psproj", bufs=3, space="PSUM") as psp:
                for w_sb, raw, cmajor in ((wq_sb, qraw, True),
                                          (wk_sb, kraw, False)):
                    kv_raw = raw[:].rearrange("p (r c) -> p r c", c=16)
                    for ot in range(8):
                        pq = psp.tile([128, RPC], f32, tag="ps")
                        for kt in range(8):
                            nc.tensor.matmul(
                                pq[:],
                                w_sb[:, kt * D + ot * 128: kt * D + ot * 128 + 128],
                                xsb[:, kt * RPC: (kt + 1) * RPC],
                                start=(kt == 0), stop=(kt == 7),
                            )
                        c0 = 2 * ot
                        if cmajor:   # Q: u = 256*c + r, contiguous copies
                            u0, u1 = 256 * c0, 256 * (c0 + 1)
                            nc.scalar.activation(raw[0:64, u0:u0 + 256],
                                                 pq[0:64, 0:256], CPY)
                            nc.vector.tensor_copy(raw[64:128, u0:u0 + 256],
                                                  pq[0:64, 256:512])
                            nc.vector.tensor_copy(raw[0:64, u1:u1 + 256],
                                                  pq[64:128, 0:256])
                            nc.scalar.activation(raw[64:128, u1:u1 + 256],
                                                 pq[64:128, 256:512], CPY)
                        else:        # K: s-ordered, strided dst (step 16)
                            nc.scalar.activation(kv_raw[0:64, 0:256, c0],
                                                 pq[0:64, 0:256], CPY)
                            nc.vector.tensor_copy(kv_raw[64:128, 0:256, c0],
                                                  pq[0:64, 256:512])
                            nc.vector.tensor_copy(kv_raw[0:64, 0:256, c0 + 1],
                                                  pq[64:128, 0:256])
                            nc.scalar.activation(kv_raw[64:128, 0:256, c0 + 1],
                                                 pq[64:128, 256:512], CPY)

                # V projection, r-stationary: vflat [r, o] natural
                vflat = vfp.tile([128, 4 * D], bf16, tag="vf")    # [p, rt*1024+o]
                for rt in range(4):
                    for ob in range(2):
                        pv = psp.tile([128, 512], f32, tag="ps")
                        for kt in range(8):
                            nc.tensor.matmul(
                                pv[:],
                                xsb[:, kt * RPC + rt * 128: kt * RPC + rt * 128 + 128],
                                wv_sb[:, kt * D + ob * 512: kt * D + ob * 512 + 512],
                                start=(kt == 0), stop=(kt == 7),
                            )
                        nc.scalar.activation(
                            vflat[:, rt * D + ob * 512: rt * D + ob * 512 + 512],
                            pv[:], CPY)

            # V reshape through DRAM: vflat [r,o] -> vsb_h[p, 65T+d] ([V|ones])
            nc.sync.dma_start(vfd.rearrange("(rt p) o -> p rt o", p=128),
                              vflat[:].rearrange("p (rt o) -> p rt o", rt=4))
            # vsb row p = plain local sk: V row s = 128*T + p
            vsbs = []
            vld = vfd.rearrange("(h T a) (c d) -> h a c T d", h=2, T=NT_SK,
                                a=8, c=16, d=DK)
            for h in range(HPC):
                vsb = vsbp.tile([128, 65 * NT_SK], bf16, tag="vsb")
                nc.gpsimd.memset(vsb[:], 1.0)   # ones col at 65T+64 survives
                dstv = vsb[:].rearrange("(a c) (T d) -> a c T d", a=8, c=16,
                                        T=NT_SK, d=65)
                for a in range(8):
                    nc.sync.dma_start(dstv[a, :, :, 0:DK], vld[h, a])
                vsbs.append(vsb)

            if dbg:
                nc.sync.dma_start(dbg_aps["dbg_qraw"][:], qraw[:])
                nc.sync.dma_start(dbg_aps["dbg_kraw"][:], kraw[:])
                nc.sync.dma_start(dbg_aps["dbg_vsb0"][:], vsbs[0][:])
                nc.sync.dma_start(dbg_aps["dbg_vsb1"][:], vsbs[1][:])
                nc.sync.dma_start(
                    dbg_aps["dbg_vflat"].rearrange("(rt p) o -> p rt o", p=128),
                    vflat[:].rearrange("p (rt o) -> p rt o", rt=4))

            # ---------- phase 2: RoPE (in place: raw tiles become rot) ----------
            for raw, c1, c2 in ((qraw, cs1_sb, cs2_sb), (kraw, cs3_sb, cs4_sb)):
                sw = ropep.tile([128, S], bf16, tag="rope")
                nc.vector.tensor_copy(sw[0:32, :], raw[32:64, :])
                nc.vector.tensor_copy(sw[32:64, :], raw[0:32, :])
                nc.vector.tensor_copy(sw[64:96, :], raw[96:128, :])
                nc.vector.tensor_copy(sw[96:128, :], raw[64:96, :])
                nc.vector.tensor_mul(sw[:], sw[:], c2[:])
                nc.vector.tensor_mul(raw[:], raw[:], c1[:])
                nc.vector.tensor_add(raw[:], raw[:], sw[:])
            qrot, krot = qraw, kraw
            if dbg:
                nc.sync.dma_start(dbg_aps["dbg_qrot"][:], qrot[:])
                nc.sync.dma_start(dbg_aps["dbg_krot"][:], krot[:])

            # ---------- phase 3: attention ----------
            outTs = []
            with (
                tc.tile_pool(name="pssc", bufs=2, space="PSUM") as pssc,
                tc.tile_pool(name="psout", bufs=2, space="PSUM") as psout,
            ):
                for h in range(HPC):
                    outT = big.tile([64, S], bf16, tag="big")
                    outTs.append(outT)
                qv = [qrot[64 * h: 64 * h + 64, :].rearrange(
                    "p (c r) -> p c r", c=16) for h in range(HPC)]
                for b in range(8):
                    nt = 4 * (b + 1)
                    outp = [psout.tile([65, 512], f32, tag="out",
                                       name=f"outp{b}_{hh}")
                            for hh in range(HPC)]
                    opv = [outp[hh][:].rearrange("p (c r) -> p c r", r=32)
                           for hh in range(HPC)]
                    slots = [(t, h) for t in range(nt) for h in range(HPC)]
                    chunks = [slots[i:i + 3] for i in range(0, len(slots), 3)]
                    for chunk in chunks:
                        ps = pssc.tile([128, 1536], f32, tag="sc")
                        pch = ppool.tile([128, 1536], bf16, tag="pp")
                        # slot layout: uniform c-major-32 groups; valid r-range
                        # [rmin, 32) per c-group, garbage at [0, rmin) (unread)
                        for j, (t, h) in enumerate(chunk):
                            rmin = 8 * max(0, t - 4 * b)   # first valid r
                            psv = ps[:, 512 * j: 512 * (j + 1)].rearrange(
                                "p (c r) -> p c r", r=32)
                            nc.tensor.matmul(
                                psv[:, :, rmin:32],
                                krot[64 * h: 64 * h + 64,
                                     128 * t: 128 * t + 128],
                                qv[h][:, :, 32 * b + rmin: 32 * (b + 1)],
                                start=True, stop=True,
                            )
                            if t >= 4 * b:   # diagonal tile: triangle mask
                                trv = psv[:, :, rmin:rmin + 8]
                                nc.vector.tensor_add(trv, trv, tri_sb[:])
                        Wtot = 512 * len(chunk)
                        nc.scalar.activation(pch[:, 0:Wtot], ps[:, 0:Wtot],
                                             EXP, scale=0.125)
                        if dbg and b == 0 and chunk[0] == (0, 0):
                            stg = ppool.tile([128, 1536], f32, tag="dbgstg",
                                             name="dbgstg")
                            nc.vector.tensor_copy(stg[:], ps[:])
                            nc.sync.dma_start(dbg_aps["dbg_ps0"][:], stg[:])
                            nc.sync.dma_start(dbg_aps["dbg_pch0"][:], pch[:])
                        for j, (t, h) in enumerate(chunk):
                            rmin = 8 * max(0, t - 4 * b)
                            pcv = pch[:, 512 * j: 512 * (j + 1)].rearrange(
                                "p (c r) -> p c r", r=32)
                            nc.tensor.matmul(
                                opv[h][:, :, rmin:32],
                                vsbs[h][:, 65 * t: 65 * t + 65],
                                pcv[:, :, rmin:32],
                                start=(t == 0), stop=(t == nt - 1),
                            )
                    # normalize block b
                    for h in range(HPC):
                        if dbg and b == 0 and h == 0:
                            stg2 = ppool.tile([65, 512], f32, tag="dbgstg",
                                              name="dbgstg2")
                            nc.vector.tensor_copy(stg2[:], outp[0][:])
                            nc.sync.dma_start(dbg_aps["dbg_outp0"][:], stg2[:])
                        dnr = misc[0:1, 64 + 512: 64 + 1024]  # raw denom row
                        dn = misc[0:1, 64: 64 + 512]
                        nc.vector.tensor_copy(dnr, outp[h][64:65, :])
                        nc.vector.reciprocal_approx_fast(out=dn, in_=dnr)
                        # broadcast dn across 64 partitions: lane-0 shuffle x2
                        nc.vector.stream_shuffle(bc_sb[0:32, :],
                                                 misc[0:32, 64:576], [0] * 32)
                        nc.vector.stream_shuffle(bc_sb[32:64, :],
                                                 misc[0:32, 64:576], [0] * 32)
                        if dbg and b == 0 and h == 0:
                            nc.sync.dma_start(dbg_aps["dbg_dn0"][:], dn)
                            nc.sync.dma_start(dbg_aps["dbg_bc0"][:], bc_sb[:])
                        # outT dst: u = 256*c + r, this block is r in [32b,32b+32)
                        osl = outTs[h][:].rearrange(
                            "p (c r) -> p c r", r=256)[:, :, 32 * b: 32 * (b + 1)]
                        nc.vector.tensor_mul(osl, outp[h][0:64, :], bc_sb[:])

            if dbg:
                nc.sync.dma_start(dbg_aps["dbg_outT0"][:], outTs[0][:])
                nc.sync.dma_start(dbg_aps["dbg_outT1"][:], outTs[1][:])

            # ---------- phase 4: output projection ----------
            y_sb = big.tile([128, 4 * D], f32, tag="big")     # [p, g*1024+o]
            with tc.tile_pool(name="psy", bufs=2, space="PSUM") as psy:
                for h in range(HPC):
                    for rt in range(2):
                        g = 2 * h + rt
                        ct = ctp.tile([128, 8 * 128], bf16, tag="ct")
                        for tp in range(8):
                            # u = 256*c + r: contiguous 128-runs per (c, rt)
                            ue = 256 * (2 * tp) + 128 * rt
                            uo = 256 * (2 * tp + 1) + 128 * rt
                            # even c -> partitions [0:64), odd -> [64:128)
                            nc.scalar.activation(
                                ct[0:64, 128 * tp: 128 * tp + 128],
                                outTs[h][:, ue:ue + 128], CPY)
                            nc.vector.tensor_copy(
                                ct[64:128, 128 * tp: 128 * tp + 128],
                                outTs[h][:, uo:uo + 128])
                        for ob in range(2):
                            py = psy.tile([128, 512], f32, tag="y")
                            for tp in range(8):
                                nc.tensor.matmul(
                                    py[:],
                                    ct[:, 128 * tp: 128 * tp + 128],
                                    wo_sb[:, tp * D + ob * 512: tp * D + ob * 512 + 512],
                                    start=(tp == 0), stop=(tp == 7),
                                )
                            nc.scalar.activation(
                                y_sb[:, g * D + ob * 512: g * D + ob * 512 + 512],
                                py[:], CPY)
            nc.sync.dma_start(y.rearrange("(g p) o -> p g o", p=128),
                              y_sb[:].rearrange("p (g o) -> p g o", g=4))

    nc.compile()
    return nc


def kernel(**inputs):
    x = np.asarray(inputs["x"], dtype=np.float32)     # [1, 4096, 1024]
    Wq = np.asarray(inputs["Wq"], dtype=np.float32)
    Wk = np.asarray(inputs["Wk"], dtype=np.float32)
    Wv = np.asarray(inputs["Wv"], dtype=np.float32)
    Wo = np.asarray(inputs["Wo"], dtype=np.float32)
    # biases are structurally zero in this problem; fold anyway if nonzero
    for bn in ("bq", "bk", "bv", "bo"):
        bv_ = np.asarray(inputs.get(bn, 0.0))
        assert np.all(bv_ == 0.0), f"{bn} nonzero: unsupported"

    from concourse.bass_utils import run_bass_kernel_spmd

    if "nc" not in _CACHE:
        _CACHE["nc"] = _build_program()
    nc = _CACHE["nc"]

    bf = ml_dtypes.bfloat16
    wqT, wkT, wvT, woT, csq1, csq2, csk1, csk2, tri = _host_arrays(
        Wq, Wk, Wv, Wo)
    shared = {"wqT": wqT, "wkT": wkT, "wvT": wvT, "woT": woT,
              "cs1": csq1, "cs2": csq2, "cs3": csk1, "cs4": csk2, "tri": tri}
    xf = x.reshape(S, D)
    in_maps = []
    for i in range(NC_N):
        xTi = np.ascontiguousarray(xf[i * RPC:(i + 1) * RPC, :].T).astype(bf)
        in_maps.append(dict(shared, xT=xTi))

    trace = bool(int(os.environ.get("BASS_KERNEL_TRACE", "0")))
    res = run_bass_kernel_spmd(nc, in_maps, core_ids=list(range(NC_N)),
                               trace=trace)
    _CACHE["last_res"] = res
    if trace and res.exec_time_ns is not None:
        print(f"HW exec time: {res.exec_time_ns} ns")
        _CACHE["exec_time_ns"] = res.exec_time_ns
        _CACHE["trace"] = res.instructions_and_trace
    out = np.concatenate([res.results[i]["y"] for i in range(NC_N)], axis=0)
    return out.reshape(1, S, D).astype(np.float32)



# revision 5
# speedup vs baseline: 1.0897x; 1.0897x over previous
"""Causal self-attention (quirky-reshape variant) on 8 TRN2 NeuronCores.

Key structural fact: the reference reshapes (B,S,H*dk) -> (B,H,S,dk) without a
transpose, so head h's Q/K/V come from rows [256h, 256h+256) of the [4096,1024]
projection output (reinterpreted as [4096,64]), and output rows [256h, 256h+256)
depend only on head h.  With 2 heads per core the problem is embarrassingly
parallel: core i consumes x rows [512i, 512i+512) + full weights and produces
output rows [512i, 512i+512).  No collectives.

v2 schedule (vs. v1 baseline):
  - Phase order: Q proj -> Q rope (DVE) || K proj (PE) -> K rope (chunked)
    -> first scores start ~31us; V proj + DRAM-roundtrip reshape overlap the
    early attention stream (deep pch buffering rides over the V wait).
  - Causal masking moved off PSUM: exp full chunks on ScalarE, then multiply
    the diagonal sub-blocks of the exp'd P by a 0/1 mask on DVE in SBUF
    (cheaper than f32 adds on PSUM, and unclogs the PSUM port for ACT).
  - Softmax normalize: reciprocal_approx_fast from PSUM -> gpsimd
    partition_broadcast (idle engine) -> single DVE multiply; double-buffered.
  - All projection PSUM->SBUF copies on ScalarE (idle during lead-in);
    ct/psy copies on DVE (4x bf16 mode).
  - Output projection emitted per (head, row-half) as soon as its 4 q-blocks
    are normalized, overlapping the attention tail; y DMA'd per 128-row group.
"""

import os

os.environ.setdefault("JAX_PLATFORMS", "cpu")

import numpy as np
import ml_dtypes

D = 1024          # d_model
H = 16            # heads
DK = 64           # head dim
S = 4096          # seq len
NC_N = 8          # cores
RPC = 512         # x rows per core
HPC = 2           # heads per core
NT_SK = 32        # sk tiles of 128 per head
ROPE_THETA = 10000.0

_CACHE = {}


def _deint_perm():
    """o' -> o source index: within each 64-block, evens first then odds."""
    d_order = list(range(0, DK, 2)) + list(range(1, DK, 2))  # position d' -> d
    perm = np.zeros(D, dtype=np.int64)
    for c in range(H):
        for dp, d in enumerate(d_order):
            perm[c * DK + dp] = c * DK + d
    return perm


def _host_arrays(Wq, Wk, Wv, Wo):
    bf = ml_dtypes.bfloat16
    perm = _deint_perm()
    wqT = np.ascontiguousarray(Wq[perm, :].T).astype(bf)   # [in, o'] deint
    wkT = np.ascontiguousarray(Wk[perm, :].T).astype(bf)
    wvT = np.ascontiguousarray(Wv.T).astype(bf)            # [in, o] natural
    woT = np.ascontiguousarray(Wo.T).astype(bf)            # [o_c, o_out]

    # RoPE tables in the [Aev, Aod, Bev, Bod] partition grouping (32 rows each;
    # identical for both heads since the angle depends only on s).
    j = np.arange(0, DK, 2, dtype=np.float64) / DK
    inv_freq = 1.0 / (ROPE_THETA ** j)                     # [32]
    # Q storage is c-major: u = 256*c + r <-> s = 16*r + c; K is s-ordered
    u = np.arange(S)
    s_of_u = 16 * (u % 256) + u // 256
    angq = np.outer(inv_freq, s_of_u)                      # [32, S] u-ordered
    angk = np.outer(inv_freq, np.arange(S))                # [32, S] s-ordered
    csq1 = np.concatenate([np.cos(angq)] * 4, 0).astype(bf)
    csq2 = np.concatenate([-np.sin(angq), np.sin(angq)] * 2, 0).astype(bf)
    csk1 = np.concatenate([np.cos(angk)] * 4, 0).astype(bf)
    csk2 = np.concatenate([-np.sin(angk), np.sin(angk)] * 2, 0).astype(bf)
    # 0/1 triangle mask applied multiplicatively to exp'd P:
    # row p = local sk (plain), col j = 8*cq + rq (c-major sq)
    p = np.arange(128)
    cq, rq = np.arange(128) // 8, np.arange(128) % 8
    sq_loc = 16 * rq + cq                                  # [128]
    tri01 = np.where(p[:, None] <= sq_loc[None, :], 1.0, 0.0).astype(bf)
    return wqT, wkT, wvT, woT, csq1, csq2, csk1, csk2, tri01


def _build_program(dbg=False):
    import concourse.bass as bass
    import concourse.tile as tile
    from concourse import bacc, mybir

    f32 = mybir.dt.float32
    bf16 = mybir.dt.bfloat16
    EXP = mybir.ActivationFunctionType.Exp
    CPY = mybir.ActivationFunctionType.Copy

    nc = bacc.Bacc("TRN2", target_bir_lowering=False, debug=False,
                   num_devices=NC_N)

    xT = nc.dram_tensor("xT", [D, RPC], bf16, kind="ExternalInput").ap()
    wq = nc.dram_tensor("wqT", [D, D], bf16, kind="ExternalInput").ap()
    wk = nc.dram_tensor("wkT", [D, D], bf16, kind="ExternalInput").ap()
    wv = nc.dram_tensor("wvT", [D, D], bf16, kind="ExternalInput").ap()
    wo = nc.dram_tensor("woT", [D, D], bf16, kind="ExternalInput").ap()
    cs1d = nc.dram_tensor("cs1", [128, S], bf16, kind="ExternalInput").ap()
    cs2d = nc.dram_tensor("cs2", [128, S], bf16, kind="ExternalInput").ap()
    cs3d = nc.dram_tensor("cs3", [128, S], bf16, kind="ExternalInput").ap()
    cs4d = nc.dram_tensor("cs4", [128, S], bf16, kind="ExternalInput").ap()
    trid = nc.dram_tensor("tri", [128, 128], bf16, kind="ExternalInput").ap()
    y = nc.dram_tensor("y", [RPC, D], f32, kind="ExternalOutput").ap()
    vfd = nc.dram_tensor("vflat_scratch", [RPC, D], bf16).ap()

    with tile.TileContext(nc) as tc:
        with (
            tc.tile_pool(name="big", bufs=3) as big,        # wq/wk/wv -> outTA/outTB/y_sb
            tc.tile_pool(name="wo", bufs=1) as wop,
            tc.tile_pool(name="xp", bufs=1) as xp,
            tc.tile_pool(name="qk", bufs=2) as qkp,          # qraw, kraw (become rot in place)
            tc.tile_pool(name="cs", bufs=4) as csp,          # 4 rope tables
            tc.tile_pool(name="sw", bufs=1) as swp,          # rope swap scratch
            tc.tile_pool(name="vf", bufs=1) as vfp,
            tc.tile_pool(name="vsb", bufs=2) as vsbp,
            tc.tile_pool(name="mask", bufs=1) as maskp,
            tc.tile_pool(name="pp", bufs=12) as ppool,       # exp'd P chunks
            tc.tile_pool(name="ct", bufs=2) as ctp,          # concatT per (h, rt)
            tc.tile_pool(name="norm", bufs=2) as normp,      # dn row + bcast block
        ):
            # ---------- phase 0: loads ----------
            xsb = xp.tile([128, 8 * RPC], bf16, tag="x")           # [p, kt*512+r]
            nc.sync.dma_start(xsb[:].rearrange("p (kt r) -> p kt r", kt=8),
                              xT.rearrange("(kt p) r -> p kt r", p=128))

            def load_w(pool, tag, src):
                t = pool.tile([128, 8 * D], bf16, tag=tag, name=f"w_{tag}")
                tv = t[:].rearrange("p (kt o) -> p kt o", kt=8)
                sv = src.rearrange("(kt p) o -> p kt o", p=128)
                for kt in range(8):
                    nc.sync.dma_start(tv[:, kt, :], sv[:, kt, :])
                return t

            wq_sb = load_w(big, "big", wq)
            wk_sb = load_w(big, "big", wk)
            cs1_sb = csp.tile([128, S], bf16, tag="cs")
            nc.sync.dma_start(cs1_sb[:], cs1d[:])
            cs2_sb = csp.tile([128, S], bf16, tag="cs")
            nc.sync.dma_start(cs2_sb[:], cs2d[:])
            cs3_sb = csp.tile([128, S], bf16, tag="cs")
            nc.sync.dma_start(cs3_sb[:], cs3d[:])
            cs4_sb = csp.tile([128, S], bf16, tag="cs")
            nc.sync.dma_start(cs4_sb[:], cs4d[:])
            wv_sb = load_w(big, "big", wv)
            wo_sb = load_w(wop, "wo", wo)
            tri_sb = maskp.tile([128, 128], bf16, tag="mask")
            nc.sync.dma_start(tri_sb[:], trid[:])

            qraw = qkp.tile([128, S], bf16, tag="qk")   # [Aev,Aod,Bev,Bod] x s
            kraw = qkp.tile([128, S], bf16, tag="qk")

            # vsb tiles early (gpsimd memset of the ones column is off-path)
            vsbs = []
            for h in range(HPC):
                vsb = vsbp.tile([128, 65 * NT_SK], bf16, tag="vsb")
                nc.gpsimd.memset(vsb[:], 1.0)   # ones col at 65T+64 survives
                vsbs.append(vsb)

            with tc.tile_pool(name="pssc", bufs=2, space="PSUM") as pssc:
                with tc.tile_pool(name="psproj", bufs=2, space="PSUM") as psp:
                    # ---------- phase 1: Q then K projection ----------
                    # (copies all on ScalarE: it is idle until the first exp)
                    for w_sb, raw, cmajor in ((wq_sb, qraw, True),
                                              (wk_sb, kraw, False)):
                        kv_raw = raw[:].rearrange("p (r c) -> p r c", c=16)
                        for ot in range(8):
                            pq = psp.tile([128, RPC], f32, tag="ps")
                            for kt in range(8):
                                nc.tensor.matmul(
                                    pq[:],
                                    w_sb[:, kt * D + ot * 128: kt * D + ot * 128 + 128],
                                    xsb[:, kt * RPC: (kt + 1) * RPC],
                                    start=(kt == 0), stop=(kt == 7),
                                )
                            c0 = 2 * ot
                            if cmajor:   # Q: u = 256*c + r, contiguous copies
                                u0, u1 = 256 * c0, 256 * (c0 + 1)
                                nc.scalar.activation(raw[0:64, u0:u0 + 256],
                                                     pq[0:64, 0:256], CPY)
                                nc.scalar.activation(raw[64:128, u0:u0 + 256],
                                                     pq[0:64, 256:512], CPY)
                                nc.scalar.activation(raw[0:64, u1:u1 + 256],
                                                     pq[64:128, 0:256], CPY)
                                nc.scalar.activation(raw[64:128, u1:u1 + 256],
                                                     pq[64:128, 256:512], CPY)
                            else:        # K: s-ordered, strided dst (step 16)
                                nc.scalar.activation(kv_raw[0:64, 0:256, c0],
                                                     pq[0:64, 0:256], CPY)
                                nc.scalar.activation(kv_raw[64:128, 0:256, c0],
                                                     pq[0:64, 256:512], CPY)
                                nc.scalar.activation(kv_raw[0:64, 0:256, c0 + 1],
                                                     pq[64:128, 0:256], CPY)
                                nc.scalar.activation(kv_raw[64:128, 0:256, c0 + 1],
                                                     pq[64:128, 256:512], CPY)

                    # ---------- phase 2: RoPE ----------
                    # Q: full width right after Q proj (DVE while K proj on PE)
                    swq = swp.tile([128, S], bf16, tag="sw", name="swq")
                    nc.vector.tensor_copy(swq[0:32, :], qraw[32:64, :])
                    nc.vector.tensor_copy(swq[32:64, :], qraw[0:32, :])
                    nc.vector.tensor_copy(swq[64:96, :], qraw[96:128, :])
                    nc.vector.tensor_copy(swq[96:128, :], qraw[64:96, :])
                    nc.vector.tensor_mul(swq[:], swq[:], cs2_sb[:])
                    nc.vector.tensor_mul(qraw[:], qraw[:], cs1_sb[:])
                    nc.vector.tensor_add(qraw[:], qraw[:], swq[:])
                    # K: chunks of 1024 cols so scores can start after chunk 0
                    for kc in range(4):
                        lo, hi = 1024 * kc, 1024 * (kc + 1)
                        swk = swp.tile([128, 1024], bf16, tag="sw",
                                       name=f"swk{kc}")
                        nc.vector.tensor_copy(swk[0:32, :], kraw[32:64, lo:hi])
                        nc.vector.tensor_copy(swk[32:64, :], kraw[0:32, lo:hi])
                        nc.vector.tensor_copy(swk[64:96, :], kraw[96:128, lo:hi])
                        nc.vector.tensor_copy(swk[96:128, :], kraw[64:96, lo:hi])
                        nc.vector.tensor_mul(swk[:], swk[:], cs4_sb[:, lo:hi])
                        nc.vector.tensor_mul(kraw[:, lo:hi], kraw[:, lo:hi],
                                             cs3_sb[:, lo:hi])
                        nc.vector.tensor_add(kraw[:, lo:hi], kraw[:, lo:hi],
                                             swk[:])
                    qrot, krot = qraw, kraw

                    # ---------- phase 3: V projection + reshape ----------
                    # rt order (0,2,1,3): both heads' first halves reach vsb
                    # early.  Reshape goes through DRAM (strided gather).
                    vflat = vfp.tile([128, 4 * D], bf16, tag="vf")
                    vfd_v = vfd.rearrange("(rt p) o -> p rt o", p=128)
                    vld = vfd.rearrange("(h T a) (c d) -> h a c T d", h=2,
                                        T=NT_SK, a=8, c=16, d=DK)
                    for rt in (0, 2, 1, 3):
                        for ob in range(2):
                            pv = psp.tile([128, 512], f32, tag="ps")
                            for kt in range(8):
                                nc.tensor.matmul(
                                    pv[:],
                                    xsb[:, kt * RPC + rt * 128: kt * RPC + rt * 128 + 128],
                                    wv_sb[:, kt * D + ob * 512: kt * D + ob * 512 + 512],
                                    start=(kt == 0), stop=(kt == 7),
                                )
                            nc.scalar.activation(
                                vflat[:, rt * D + ob * 512: rt * D + ob * 512 + 512],
                                pv[:], CPY)
                        nc.sync.dma_start(vfd_v[:, rt, :],
                                          vflat[:].rearrange(
                                              "p (rt o) -> p rt o", rt=4)[:, rt, :])
                        # quirky map: vfd rows [256h+128*half, +128) = head h,
                        # tiles T in [16*half, 16*half+16)
                        h, half = divmod(rt, 2)
                        T0 = 16 * half
                        dstv = vsbs[h][:].rearrange("(a c) (T d) -> a c T d",
                                                    a=8, c=16, T=NT_SK, d=65)
                        for a in range(8):
                            nc.sync.dma_start(dstv[a, :, T0:T0 + 16, 0:DK],
                                              vld[h, a, :, T0:T0 + 16, :])

                # ---------- phase 4: attention ----------
                with tc.tile_pool(name="psout", bufs=2, space="PSUM") as psout:
                    outTs = []
                    for h in range(HPC):
                        outT = big.tile([64, S], bf16, tag="big")
                        outTs.append(outT)
                    y_sb = None
                    qv = [qrot[64 * h: 64 * h + 64, :].rearrange(
                        "p (c r) -> p c r", c=16) for h in range(HPC)]
                    triv = tri_sb[:].rearrange("p (c r) -> p c r", r=8)
                    yv = y.rearrange("(g p) o -> p g o", p=128)

                    def emit_oproj(h, rt, y_sb):
                        g = 2 * h + rt
                        ct = ctp.tile([128, 8 * 128], bf16, tag="ct")
                        for tp in range(8):
                            ue = 256 * (2 * tp) + 128 * rt
                            uo = 256 * (2 * tp + 1) + 128 * rt
                            nc.vector.tensor_copy(
                                ct[0:64, 128 * tp: 128 * tp + 128],
                                outTs[h][:, ue:ue + 128])
                            nc.vector.tensor_copy(
                                ct[64:128, 128 * tp: 128 * tp + 128],
                                outTs[h][:, uo:uo + 128])
                        for ob in range(2):
                            py = psout.tile([128, 512], f32, tag="out",
                                            name=f"py{g}_{ob}")
                            for tp in range(8):
                                nc.tensor.matmul(
                                    py[:],
                                    ct[:, 128 * tp: 128 * tp + 128],
                                    wo_sb[:, tp * D + ob * 512: tp * D + ob * 512 + 512],
                                    start=(tp == 0), stop=(tp == 7),
                                )
                            nc.vector.tensor_copy(
                                y_sb[:, g * D + ob * 512: g * D + ob * 512 + 512],
                                py[:])
                        nc.sync.dma_start(yv[:, g, :], y_sb[:, g * D:(g + 1) * D])

                    for b in range(8):
                        nt = 4 * (b + 1)
                        outp = [psout.tile([65, 512], f32, tag="out",
                                           name=f"outp{b}_{hh}")
                                for hh in range(HPC)]
                        opv = [outp[hh][:].rearrange("p (c r) -> p c r", r=32)
                               for hh in range(HPC)]
                        slots = [(t, h) for t in range(nt) for h in range(HPC)]
                        chunks = [slots[i:i + 3] for i in range(0, len(slots), 3)]
                        for chunk in chunks:
                            ps = pssc.tile([128, 1536], f32, tag="sc")
                            pch = ppool.tile([128, 1536], bf16, tag="pp")
                            # slot layout: uniform c-major-32 groups; valid
                            # r-range [rmin, 32) per c-group, garbage at
                            # [0, rmin) (never read downstream)
                            for j, (t, h) in enumerate(chunk):
                                rmin = 8 * max(0, t - 4 * b)
                                psv = ps[:, 512 * j: 512 * (j + 1)].rearrange(
                                    "p (c r) -> p c r", r=32)
                                nc.tensor.matmul(
                                    psv[:, :, rmin:32],
                                    krot[64 * h: 64 * h + 64,
                                         128 * t: 128 * t + 128],
                                    qv[h][:, :, 32 * b + rmin: 32 * (b + 1)],
                                    start=True, stop=True,
                                )
                            Wtot = 512 * len(chunk)
                            nc.scalar.activation(pch[:, 0:Wtot], ps[:, 0:Wtot],
                                                 EXP, scale=0.125)
                            # diagonal tiles: zero the upper triangle of the
                            # exp'd P (multiplicative 0/1 mask, SBUF bf16)
                            for j, (t, h) in enumerate(chunk):
                                if t >= 4 * b:
                                    rmin = 8 * (t - 4 * b)
                                    pm = pch[:, 512 * j: 512 * (j + 1)].rearrange(
                                        "p (c r) -> p c r", r=32)[:, :, rmin:rmin + 8]
                                    nc.vector.tensor_mul(pm, pm, triv)
                            for j, (t, h) in enumerate(chunk):
                                rmin = 8 * max(0, t - 4 * b)
                                pcv = pch[:, 512 * j: 512 * (j + 1)].rearrange(
                                    "p (c r) -> p c r", r=32)
                                nc.tensor.matmul(
                                    opv[h][:, :, rmin:32],
                                    vsbs[h][:, 65 * t: 65 * t + 65],
                                    pcv[:, :, rmin:32],
                                    start=(t == 0), stop=(t == nt - 1),
                                )
                        # normalize block b: row 64 of outp holds the softmax
                        # denominators (ones column of [V|1] stationary)
                        for h in range(HPC):
                            nrm = normp.tile([128, 1024], f32, tag="norm",
                                             name=f"nrm{b}_{h}")
                            nc.vector.tensor_copy(nrm[0:1, 512:1024],
                                                  outp[h][64:65, :])
                            nc.vector.reciprocal_approx_fast(
                                out=nrm[0:1, 0:512], in_=nrm[0:1, 512:1024])
                            nc.vector.stream_shuffle(nrm[64:96, 0:512],
                                                     nrm[0:32, 0:512], [0] * 32)
                            nc.vector.stream_shuffle(nrm[96:128, 0:512],
                                                     nrm[0:32, 0:512], [0] * 32)
                            osl = outTs[h][:].rearrange(
                                "p (c r) -> p c r", r=256)[:, :, 32 * b: 32 * (b + 1)]
                            nc.vector.tensor_mul(osl, outp[h][0:64, :],
                                                 nrm[64:128, 0:512])
                        # ---------- phase 5: output projection pieces ----------
                        if b == 3:
                            y_sb = big.tile([128, 4 * D], f32, tag="big")
                            for h in range(HPC):
                                emit_oproj(h, 0, y_sb)
                        elif b == 7:
                            for h in range(HPC):
                                emit_oproj(h, 1, y_sb)

    nc.compile()
    return nc


def kernel(**inputs):
    x = np.asarray(inputs["x"], dtype=np.float32)     # [1, 4096, 1024]
    Wq = np.asarray(inputs["Wq"], dtype=np.float32)
    Wk = np.asarray(inputs["Wk"], dtype=np.float32)
    Wv = np.asarray(inputs["Wv"], dtype=np.float32)
    Wo = np.asarray(inputs["Wo"], dtype=np.float32)
    # biases are structurally zero in this problem; fold anyway if nonzero
    for bn in ("bq", "bk", "bv", "bo"):
        bv_ = np.asarray(inputs.get(bn, 0.0))
        assert np.all(bv_ == 0.0), f"{bn} nonzero: unsupported"

    from concourse.bass_utils import run_bass_kernel_spmd

    if "nc" not in _CACHE:
        _CACHE["nc"] = _build_program()
    nc = _CACHE["nc"]

    bf = ml_dtypes.bfloat16
    wqT, wkT, wvT, woT, csq1, csq2, csk1, csk2, tri01 = _host_arrays(
        Wq, Wk, Wv, Wo)
    shared = {"wqT": wqT, "wkT": wkT, "wvT": wvT, "woT": woT,
              "cs1": csq1, "cs2": csq2, "cs3": csk1, "cs4": csk2,
              "tri": tri01}
    xf = x.reshape(S, D)
    in_maps = []
    for i in range(NC_N):
        xTi = np.ascontiguousarray(xf[i * RPC:(i + 1) * RPC, :].T).astype(bf)
        in_maps.append(dict(shared, xT=xTi))

    trace = bool(int(os.environ.get("BASS_KERNEL_TRACE", "0")))
    res = run_bass_kernel_spmd(nc, in_maps, core_ids=list(range(NC_N)),
                               trace=trace)
    _CACHE["last_res"] = res
    if trace and res.exec_time_ns is not None:
        print(f"HW exec time: {res.exec_time_ns} ns")
        _CACHE["exec_time_ns"] = res.exec_time_ns
        _CACHE["trace"] = res.instructions_and_trace
    out = np.concatenate([res.results[i]["y"] for i in range(NC_N)], axis=0)
    return out.reshape(1, S, D).astype(np.float32)


# revision 12
# speedup vs baseline: 1.1047x; 1.0137x over previous
"""Causal self-attention (quirky-reshape variant) on 8 TRN2 NeuronCores.

Key structural fact: the reference reshapes (B,S,H*dk) -> (B,H,S,dk) without a
transpose, so head h's Q/K/V come from rows [256h, 256h+256) of the [4096,1024]
projection output (reinterpreted as [4096,64]), and output rows [256h, 256h+256)
depend only on head h.  With 2 heads per core the problem is embarrassingly
parallel: core i consumes x rows [512i, 512i+512) + full weights and produces
output rows [512i, 512i+512).  No collectives.

v2 schedule (vs. v1 baseline):
  - Phase order: Q proj -> Q rope (DVE) || K proj (PE) -> K rope (chunked)
    -> first scores start ~31us; V proj + DRAM-roundtrip reshape overlap the
    early attention stream (deep pch buffering rides over the V wait).
  - Causal masking moved off PSUM: exp full chunks on ScalarE, then multiply
    the diagonal sub-blocks of the exp'd P by a 0/1 mask on DVE in SBUF
    (cheaper than f32 adds on PSUM, and unclogs the PSUM port for ACT).
  - Softmax normalize: reciprocal_approx_fast from PSUM -> gpsimd
    partition_broadcast (idle engine) -> single DVE multiply; double-buffered.
  - All projection PSUM->SBUF copies on ScalarE (idle during lead-in);
    ct/psy copies on DVE (4x bf16 mode).
  - Output projection emitted per (head, row-half) as soon as its 4 q-blocks
    are normalized, overlapping the attention tail; y DMA'd per 128-row group.
"""

import os

os.environ.setdefault("JAX_PLATFORMS", "cpu")

import numpy as np
import ml_dtypes

D = 1024          # d_model
H = 16            # heads
DK = 64           # head dim
S = 4096          # seq len
NC_N = 8          # cores
RPC = 512         # x rows per core
HPC = 2           # heads per core
NT_SK = 32        # sk tiles of 128 per head
ROPE_THETA = 10000.0

_CACHE = {}


def _deint_perm():
    """o' -> o source index: within each 64-block, evens first then odds."""
    d_order = list(range(0, DK, 2)) + list(range(1, DK, 2))  # position d' -> d
    perm = np.zeros(D, dtype=np.int64)
    for c in range(H):
        for dp, d in enumerate(d_order):
            perm[c * DK + dp] = c * DK + d
    return perm


def _host_arrays(Wq, Wk, Wv, Wo):
    bf = ml_dtypes.bfloat16
    perm = _deint_perm()
    wqT = np.ascontiguousarray(Wq[perm, :].T).astype(bf)   # [in, o'] deint
    wkT = np.ascontiguousarray(Wk[perm, :].T).astype(bf)
    wvT = np.ascontiguousarray(Wv.T).astype(bf)            # [in, o] natural
    woT = np.ascontiguousarray(Wo.T).astype(bf)            # [o_c, o_out]

    # RoPE tables in the [Aev, Aod, Bev, Bod] partition grouping (32 rows each;
    # identical for both heads since the angle depends only on s).
    j = np.arange(0, DK, 2, dtype=np.float64) / DK
    inv_freq = 1.0 / (ROPE_THETA ** j)                     # [32]
    # Q storage is c-major: u = 256*c + r <-> s = 16*r + c; K is s-ordered
    u = np.arange(S)
    s_of_u = 16 * (u % 256) + u // 256
    angq = np.outer(inv_freq, s_of_u)                      # [32, S] u-ordered
    angk = np.outer(inv_freq, np.arange(S))                # [32, S] s-ordered
    csq1 = np.concatenate([np.cos(angq)] * 4, 0).astype(bf)
    csq2 = np.concatenate([-np.sin(angq), np.sin(angq)] * 2, 0).astype(bf)
    csk1 = np.concatenate([np.cos(angk)] * 4, 0).astype(bf)
    csk2 = np.concatenate([-np.sin(angk), np.sin(angk)] * 2, 0).astype(bf)
    # 0/1 triangle mask applied multiplicatively to exp'd P:
    # row p = local sk (plain), col j = 8*cq + rq (c-major sq)
    p = np.arange(128)
    cq, rq = np.arange(128) // 8, np.arange(128) % 8
    sq_loc = 16 * rq + cq                                  # [128]
    tri01 = np.where(p[:, None] <= sq_loc[None, :], 1.0, 0.0).astype(bf)
    return wqT, wkT, wvT, woT, csq1, csq2, csk1, csk2, tri01


def _build_program(dbg=False):
    import concourse.bass as bass
    import concourse.tile as tile
    from concourse import bacc, mybir

    f32 = mybir.dt.float32
    bf16 = mybir.dt.bfloat16
    EXP = mybir.ActivationFunctionType.Exp
    CPY = mybir.ActivationFunctionType.Copy

    nc = bacc.Bacc("TRN2", target_bir_lowering=False, debug=False,
                   num_devices=NC_N)

    xT = nc.dram_tensor("xT", [D, RPC], bf16, kind="ExternalInput").ap()
    wq = nc.dram_tensor("wqT", [D, D], bf16, kind="ExternalInput").ap()
    wk = nc.dram_tensor("wkT", [D, D], bf16, kind="ExternalInput").ap()
    wv = nc.dram_tensor("wvT", [D, D], bf16, kind="ExternalInput").ap()
    wo = nc.dram_tensor("woT", [D, D], bf16, kind="ExternalInput").ap()
    cs1d = nc.dram_tensor("cs1", [128, S], bf16, kind="ExternalInput").ap()
    cs2d = nc.dram_tensor("cs2", [128, S], bf16, kind="ExternalInput").ap()
    cs3d = nc.dram_tensor("cs3", [128, S], bf16, kind="ExternalInput").ap()
    cs4d = nc.dram_tensor("cs4", [128, S], bf16, kind="ExternalInput").ap()
    trid = nc.dram_tensor("tri", [128, 128], bf16, kind="ExternalInput").ap()
    y = nc.dram_tensor("y", [RPC, D], f32, kind="ExternalOutput").ap()
    vfd = nc.dram_tensor("vflat_scratch", [RPC, D], bf16).ap()

    with tile.TileContext(nc) as tc:
        with (
            tc.tile_pool(name="big", bufs=3) as big,        # wq/wk/wv -> outTA/outTB/y_sb
            tc.tile_pool(name="wo", bufs=1) as wop,
            tc.tile_pool(name="xp", bufs=1) as xp,
            tc.tile_pool(name="qk", bufs=2) as qkp,          # qraw, kraw (become rot in place)
            tc.tile_pool(name="cs", bufs=4) as csp,          # 4 rope tables
            tc.tile_pool(name="sw", bufs=1) as swp,          # rope swap scratch
            tc.tile_pool(name="vf", bufs=1) as vfp,
            tc.tile_pool(name="vsb", bufs=2) as vsbp,
            tc.tile_pool(name="mask", bufs=1) as maskp,
            tc.tile_pool(name="pp", bufs=18) as ppool,       # exp'd P chunks
            tc.tile_pool(name="ct", bufs=2) as ctp,          # concatT per (h, rt)
            tc.tile_pool(name="norm", bufs=2) as normp,      # dn row + bcast block
        ):
            # ---------- phase 0: loads ----------
            xsb = xp.tile([128, 8 * RPC], bf16, tag="x")           # [p, kt*512+r]
            xv = xsb[:].rearrange("p (kt r) -> p kt r", kt=8)
            xs = xT.rearrange("(kt p) r -> p kt r", p=128)
            for kt in range(8):
                nc.sync.dma_start(xv[:, kt, :], xs[:, kt, :])

            def load_w(pool, tag, src):
                t = pool.tile([128, 8 * D], bf16, tag=tag, name=f"w_{tag}")
                tv = t[:].rearrange("p (kt o) -> p kt o", kt=8)
                sv = src.rearrange("(kt p) o -> p kt o", p=128)
                for kt in range(8):
                    nc.sync.dma_start(tv[:, kt, :], sv[:, kt, :])
                return t

            wk_sb = load_w(big, "big", wk)
            wq_sb = load_w(big, "big", wq)
            cs1_sb = csp.tile([128, S], bf16, tag="cs")
            nc.sync.dma_start(cs1_sb[:], cs1d[:])
            cs2_sb = csp.tile([128, S], bf16, tag="cs")
            nc.sync.dma_start(cs2_sb[:], cs2d[:])
            cs3_sb = csp.tile([128, S], bf16, tag="cs")
            nc.sync.dma_start(cs3_sb[:], cs3d[:])
            cs4_sb = csp.tile([128, S], bf16, tag="cs")
            nc.sync.dma_start(cs4_sb[:], cs4d[:])
            wv_sb = load_w(big, "big", wv)
            wo_sb = load_w(wop, "wo", wo)
            tri_sb = maskp.tile([128, 128], bf16, tag="mask")
            nc.sync.dma_start(tri_sb[:], trid[:])

            qraw = qkp.tile([128, S], bf16, tag="qk")   # [Aev,Aod,Bev,Bod] x s
            kraw = qkp.tile([128, S], bf16, tag="qk")

            # vsb tiles early (gpsimd memset of the ones column is off-path)
            vsbs = []
            for h in range(HPC):
                vsb = vsbp.tile([128, 65 * NT_SK], bf16, tag="vsb")
                nc.gpsimd.memset(vsb[:], 1.0)   # ones col at 65T+64 survives
                vsbs.append(vsb)

            with tc.tile_pool(name="pssc", bufs=2, space="PSUM") as pssc:
                with tc.tile_pool(name="psproj", bufs=2, space="PSUM") as psp:
                    # ---------- phase 1a: K projection (copies on DVE — the
                    # strided dst is cheap there; ScalarE pays 3x for it) ----
                    kv_raw = kraw[:].rearrange("p (r c) -> p r c", c=16)
                    for ot in range(8):
                        pq = psp.tile([128, RPC], f32, tag="ps")
                        for kt in range(8):
                            nc.tensor.matmul(
                                pq[:],
                                wk_sb[:, kt * D + ot * 128: kt * D + ot * 128 + 128],
                                xsb[:, kt * RPC: (kt + 1) * RPC],
                                start=(kt == 0), stop=(kt == 7),
                            )
                        c0 = 2 * ot
                        nc.vector.tensor_copy(kv_raw[0:64, 0:256, c0],
                                              pq[0:64, 0:256])
                        nc.vector.tensor_copy(kv_raw[64:128, 0:256, c0],
                                              pq[0:64, 256:512])
                        nc.vector.tensor_copy(kv_raw[0:64, 0:256, c0 + 1],
                                              pq[64:128, 0:256])
                        nc.vector.tensor_copy(kv_raw[64:128, 0:256, c0 + 1],
                                              pq[64:128, 256:512])
                    # K rope in s-chunks of 1024 (chunk c covers sk tiles
                    # 8c..8c+7); scores for b=0 start after chunk 0
                    for kc in range(4):
                        lo, hi = 1024 * kc, 1024 * (kc + 1)
                        swk = swp.tile([128, S], bf16, tag="sw",
                                       name=f"swk{kc}")
                        nc.vector.tensor_copy(swk[0:32, 0:1024],
                                              kraw[32:64, lo:hi])
                        nc.vector.tensor_copy(swk[32:64, 0:1024],
                                              kraw[0:32, lo:hi])
                        nc.vector.tensor_copy(swk[64:96, 0:1024],
                                              kraw[96:128, lo:hi])
                        nc.vector.tensor_copy(swk[96:128, 0:1024],
                                              kraw[64:96, lo:hi])
                        nc.vector.tensor_mul(swk[:, 0:1024], swk[:, 0:1024],
                                             cs4_sb[:, lo:hi])
                        nc.vector.tensor_mul(kraw[:, lo:hi], kraw[:, lo:hi],
                                             cs3_sb[:, lo:hi])
                        nc.vector.tensor_add(kraw[:, lo:hi], kraw[:, lo:hi],
                                             swk[:, 0:1024])

                    # ---------- phase 1b: Q projection (copies on ScalarE:
                    # contiguous, and DVE is busy with K rope) ----------
                    for ot in range(8):
                        pq = psp.tile([128, RPC], f32, tag="ps")
                        for kt in range(8):
                            nc.tensor.matmul(
                                pq[:],
                                wq_sb[:, kt * D + ot * 128: kt * D + ot * 128 + 128],
                                xsb[:, kt * RPC: (kt + 1) * RPC],
                                start=(kt == 0), stop=(kt == 7),
                            )
                        c0 = 2 * ot
                        u0, u1 = 256 * c0, 256 * (c0 + 1)
                        nc.scalar.activation(qraw[0:64, u0:u0 + 256],
                                             pq[0:64, 0:256], CPY)
                        nc.scalar.activation(qraw[64:128, u0:u0 + 256],
                                             pq[0:64, 256:512], CPY)
                        nc.scalar.activation(qraw[0:64, u1:u1 + 256],
                                             pq[64:128, 0:256], CPY)
                        nc.scalar.activation(qraw[64:128, u1:u1 + 256],
                                             pq[64:128, 256:512], CPY)
                    # Q rope, full width
                    swq = swp.tile([128, S], bf16, tag="sw", name="swq")
                    nc.vector.tensor_copy(swq[0:32, :], qraw[32:64, :])
                    nc.vector.tensor_copy(swq[32:64, :], qraw[0:32, :])
                    nc.vector.tensor_copy(swq[64:96, :], qraw[96:128, :])
                    nc.vector.tensor_copy(swq[96:128, :], qraw[64:96, :])
                    nc.vector.tensor_mul(swq[:], swq[:], cs2_sb[:])
                    nc.vector.tensor_mul(qraw[:], qraw[:], cs1_sb[:])
                    nc.vector.tensor_add(qraw[:], qraw[:], swq[:])
                    qrot, krot = qraw, kraw

                    # ---------- phase 3: V projection + reshape ----------
                    # rt order (0,2,1,3): both heads' first halves reach vsb
                    # early.  Reshape goes through DRAM (strided gather).
                    vflat = vfp.tile([128, 4 * D], bf16, tag="vf")
                    vfd_v = vfd.rearrange("(rt p) o -> p rt o", p=128)
                    vld = vfd.rearrange("(h T a) (c d) -> h a c T d", h=2,
                                        T=NT_SK, a=8, c=16, d=DK)
                    for rt in (0, 2, 1, 3):
                        for ob in range(2):
                            pv = psp.tile([128, 512], f32, tag="ps")
                            for kt in range(8):
                                nc.tensor.matmul(
                                    pv[:],
                                    xsb[:, kt * RPC + rt * 128: kt * RPC + rt * 128 + 128],
                                    wv_sb[:, kt * D + ob * 512: kt * D + ob * 512 + 512],
                                    start=(kt == 0), stop=(kt == 7),
                                )
                            nc.scalar.activation(
                                vflat[:, rt * D + ob * 512: rt * D + ob * 512 + 512],
                                pv[:], CPY)
                        nc.sync.dma_start(vfd_v[:, rt, :],
                                          vflat[:].rearrange(
                                              "p (rt o) -> p rt o", rt=4)[:, rt, :])
                        # quirky map: vfd rows [256h+128*half, +128) = head h,
                        # tiles T in [16*half, 16*half+16)
                        h, half = divmod(rt, 2)
                        T0 = 16 * half
                        dstv = vsbs[h][:].rearrange("(a c) (T d) -> a c T d",
                                                    a=8, c=16, T=NT_SK, d=65)
                        for a in range(8):
                            nc.sync.dma_start(dstv[a, :, T0:T0 + 16, 0:DK],
                                              vld[h, a, :, T0:T0 + 16, :])

                # ---------- phase 4: attention ----------
                with tc.tile_pool(name="psout", bufs=4, space="PSUM") as psout:
                    outTs = []
                    for h in range(HPC):
                        outT = big.tile([64, S], bf16, tag="big")
                        outTs.append(outT)
                    y_sb = None
                    qv = [qrot[64 * h: 64 * h + 64, :].rearrange(
                        "p (c r) -> p c r", c=16) for h in range(HPC)]
                    triv = tri_sb[:].rearrange("p (c r) -> p c r", r=8)
                    yv = y.rearrange("(g p) o -> p g o", p=128)

                    def emit_oproj(h, rt, y_sb):
                        g = 2 * h + rt
                        ct = ctp.tile([128, 8 * 128], bf16, tag="ct")
                        for tp in range(8):
                            ue = 256 * (2 * tp) + 128 * rt
                            uo = 256 * (2 * tp + 1) + 128 * rt
                            nc.vector.tensor_copy(
                                ct[0:64, 128 * tp: 128 * tp + 128],
                                outTs[h][:, ue:ue + 128])
                            nc.vector.tensor_copy(
                                ct[64:128, 128 * tp: 128 * tp + 128],
                                outTs[h][:, uo:uo + 128])
                        for ob in range(2):
                            py = psout.tile([128, 512], f32, tag="out",
                                            name=f"py{g}_{ob}")
                            for tp in range(8):
                                nc.tensor.matmul(
                                    py[:],
                                    ct[:, 128 * tp: 128 * tp + 128],
                                    wo_sb[:, tp * D + ob * 512: tp * D + ob * 512 + 512],
                                    start=(tp == 0), stop=(tp == 7),
                                )
                            nc.vector.tensor_copy(
                                y_sb[:, g * D + ob * 512: g * D + ob * 512 + 512],
                                py[:])
                        nc.sync.dma_start(yv[:, g, :], y_sb[:, g * D:(g + 1) * D])

                    for b in range(8):
                        nt = 4 * (b + 1)
                        outp = [psout.tile([65, 512], f32, tag="out",
                                           name=f"outp{b}_{hh}")
                                for hh in range(HPC)]
                        opv = [outp[hh][:].rearrange("p (c r) -> p c r", r=32)
                               for hh in range(HPC)]
                        slots = [(t, h) for t in range(nt) for h in range(HPC)]
                        chunks = [slots[i:i + 2] for i in range(0, len(slots), 2)]
                        for chunk in chunks:
                            ps = pssc.tile([128, 1024], f32, tag="sc")
                            pch = ppool.tile([128, 1024], bf16, tag="pp")
                            # slot layout: uniform c-major-32 groups; valid
                            # r-range [rmin, 32) per c-group, garbage at
                            # [0, rmin) (never read downstream)
                            for j, (t, h) in enumerate(chunk):
                                rmin = 8 * max(0, t - 4 * b)
                                psv = ps[:, 512 * j: 512 * (j + 1)].rearrange(
                                    "p (c r) -> p c r", r=32)
                                nc.tensor.matmul(
                                    psv[:, :, rmin:32],
                                    krot[64 * h: 64 * h + 64,
                                         128 * t: 128 * t + 128],
                                    qv[h][:, :, 32 * b + rmin: 32 * (b + 1)],
                                    start=True, stop=True,
                                )
                            Wtot = 512 * len(chunk)
                            nc.scalar.activation(pch[:, 0:Wtot], ps[:, 0:Wtot],
                                                 EXP, scale=0.125)
                            # diagonal tiles: zero the upper triangle of the
                            # exp'd P (multiplicative 0/1 mask, SBUF bf16)
                            for j, (t, h) in enumerate(chunk):
                                if t >= 4 * b:
                                    rmin = 8 * (t - 4 * b)
                                    pm = pch[:, 512 * j: 512 * (j + 1)].rearrange(
                                        "p (c r) -> p c r", r=32)[:, :, rmin:rmin + 8]
                                    nc.vector.tensor_mul(pm, pm, triv)
                            for j, (t, h) in enumerate(chunk):
                                rmin = 8 * max(0, t - 4 * b)
                                pcv = pch[:, 512 * j: 512 * (j + 1)].rearrange(
                                    "p (c r) -> p c r", r=32)
                                nc.tensor.matmul(
                                    opv[h][:, :, rmin:32],
                                    vsbs[h][:, 65 * t: 65 * t + 65],
                                    pcv[:, :, rmin:32],
                                    start=(t == 0), stop=(t == nt - 1),
                                )
                        # normalize block b: row 64 of outp holds the softmax
                        # denominators (ones column of [V|1] stationary)
                        for h in range(HPC):
                            nrm = normp.tile([128, 1024], f32, tag="norm",
                                             name=f"nrm{b}_{h}")
                            nc.vector.tensor_copy(nrm[0:1, 512:1024],
                                                  outp[h][64:65, :])
                            nc.vector.reciprocal_approx_fast(
                                out=nrm[0:1, 0:512], in_=nrm[0:1, 512:1024])
                            nc.vector.stream_shuffle(nrm[64:96, 0:512],
                                                     nrm[0:32, 0:512], [0] * 32)
                            nc.vector.stream_shuffle(nrm[96:128, 0:512],
                                                     nrm[0:32, 0:512], [0] * 32)
                            osl = outTs[h][:].rearrange(
                                "p (c r) -> p c r", r=256)[:, :, 32 * b: 32 * (b + 1)]
                            nc.vector.tensor_mul(osl, outp[h][0:64, :],
                                                 nrm[64:128, 0:512])
                            # ---------- phase 5: output projection pieces ----
                            # (h, rt) emitted as soon as its 4 q-blocks are
                            # normalized; at b==7 interleave with the other
                            # head's normalize to shorten the tail
                            if b == 3 and h == HPC - 1:
                                y_sb = big.tile([128, 4 * D], f32, tag="big")
                                for hh in range(HPC):
                                    emit_oproj(hh, 0, y_sb)
                            elif b == 7:
                                emit_oproj(h, 1, y_sb)

    nc.compile()
    return nc


def kernel(**inputs):
    x = np.asarray(inputs["x"], dtype=np.float32)     # [1, 4096, 1024]
    Wq = np.asarray(inputs["Wq"], dtype=np.float32)
    Wk = np.asarray(inputs["Wk"], dtype=np.float32)
    Wv = np.asarray(inputs["Wv"], dtype=np.float32)
    Wo = np.asarray(inputs["Wo"], dtype=np.float32)
    # biases are structurally zero in this problem; fold anyway if nonzero
    for bn in ("bq", "bk", "bv", "bo"):
        bv_ = np.asarray(inputs.get(bn, 0.0))
        assert np.all(bv_ == 0.0), f"{bn} nonzero: unsupported"

    from concourse.bass_utils import run_bass_kernel_spmd

    if "nc" not in _CACHE:
        _CACHE["nc"] = _build_program()
    nc = _CACHE["nc"]

    bf = ml_dtypes.bfloat16
    wqT, wkT, wvT, woT, csq1, csq2, csk1, csk2, tri01 = _host_arrays(
        Wq, Wk, Wv, Wo)
    shared = {"wqT": wqT, "wkT": wkT, "wvT": wvT, "woT": woT,
              "cs1": csq1, "cs2": csq2, "cs3": csk1, "cs4": csk2,
              "tri": tri01}
    xf = x.reshape(S, D)
    in_maps = []
    for i in range(NC_N):
        xTi = np.ascontiguousarray(xf[i * RPC:(i + 1) * RPC, :].T).astype(bf)
        in_maps.append(dict(shared, xT=xTi))

    trace = bool(int(os.environ.get("BASS_KERNEL_TRACE", "0")))
    res = run_bass_kernel_spmd(nc, in_maps, core_ids=list(range(NC_N)),
                               trace=trace)
    _CACHE["last_res"] = res
    if trace and res.exec_time_ns is not None:
        print(f"HW exec time: {res.exec_time_ns} ns")
        _CACHE["exec_time_ns"] = res.exec_time_ns
        _CACHE["trace"] = res.instructions_and_trace
    out = np.concatenate([res.results[i]["y"] for i in range(NC_N)], axis=0)
    return out.reshape(1, S, D).astype(np.float32)


# revision 17
# speedup vs baseline: 1.1143x; 1.0087x over previous
"""Causal self-attention (quirky-reshape variant) on 8 TRN2 NeuronCores.

Key structural fact: the reference reshapes (B,S,H*dk) -> (B,H,S,dk) without a
transpose, so head h's Q/K/V come from rows [256h, 256h+256) of the [4096,1024]
projection output (reinterpreted as [4096,64]), and output rows [256h, 256h+256)
depend only on head h.  With 2 heads per core the problem is embarrassingly
parallel: core i consumes x rows [512i, 512i+512) + full weights and produces
output rows [512i, 512i+512).  No collectives.

v2 schedule (vs. v1 baseline):
  - Phase order: Q proj -> Q rope (DVE) || K proj (PE) -> K rope (chunked)
    -> first scores start ~31us; V proj + DRAM-roundtrip reshape overlap the
    early attention stream (deep pch buffering rides over the V wait).
  - Causal masking moved off PSUM: exp full chunks on ScalarE, then multiply
    the diagonal sub-blocks of the exp'd P by a 0/1 mask on DVE in SBUF
    (cheaper than f32 adds on PSUM, and unclogs the PSUM port for ACT).
  - Softmax normalize: reciprocal_approx_fast from PSUM -> gpsimd
    partition_broadcast (idle engine) -> single DVE multiply; double-buffered.
  - All projection PSUM->SBUF copies on ScalarE (idle during lead-in);
    ct/psy copies on DVE (4x bf16 mode).
  - Output projection emitted per (head, row-half) as soon as its 4 q-blocks
    are normalized, overlapping the attention tail; y DMA'd per 128-row group.
"""

import os

os.environ.setdefault("JAX_PLATFORMS", "cpu")

import numpy as np
import ml_dtypes

D = 1024          # d_model
H = 16            # heads
DK = 64           # head dim
S = 4096          # seq len
NC_N = 8          # cores
RPC = 512         # x rows per core
HPC = 2           # heads per core
NT_SK = 32        # sk tiles of 128 per head
ROPE_THETA = 10000.0

_CACHE = {}


def _deint_perm():
    """o' -> o source index: within each 64-block, evens first then odds."""
    d_order = list(range(0, DK, 2)) + list(range(1, DK, 2))  # position d' -> d
    perm = np.zeros(D, dtype=np.int64)
    for c in range(H):
        for dp, d in enumerate(d_order):
            perm[c * DK + dp] = c * DK + d
    return perm


def _host_arrays(Wq, Wk, Wv, Wo):
    bf = ml_dtypes.bfloat16
    perm = _deint_perm()
    wqT = np.ascontiguousarray(Wq[perm, :].T).astype(bf)   # [in, o'] deint
    wkT = np.ascontiguousarray(Wk[perm, :].T).astype(bf)
    wvT = np.ascontiguousarray(Wv.T).astype(bf)            # [in, o] natural
    woT = np.ascontiguousarray(Wo.T).astype(bf)            # [o_c, o_out]

    # RoPE tables in the [Aev, Aod, Bev, Bod] partition grouping (32 rows each;
    # identical for both heads since the angle depends only on s).
    j = np.arange(0, DK, 2, dtype=np.float64) / DK
    inv_freq = 1.0 / (ROPE_THETA ** j)                     # [32]
    # Q storage is c-major: u = 256*c + r <-> s = 16*r + c; K is s-ordered
    u = np.arange(S)
    s_of_u = 16 * (u % 256) + u // 256
    angq = np.outer(inv_freq, s_of_u)                      # [32, S] u-ordered
    angk = np.outer(inv_freq, np.arange(S))                # [32, S] s-ordered
    csq1 = np.concatenate([np.cos(angq)] * 4, 0).astype(bf)
    csq2 = np.concatenate([-np.sin(angq), np.sin(angq)] * 2, 0).astype(bf)
    csk1 = np.concatenate([np.cos(angk)] * 4, 0).astype(bf)
    csk2 = np.concatenate([-np.sin(angk), np.sin(angk)] * 2, 0).astype(bf)
    # 0/1 triangle mask applied multiplicatively to exp'd P:
    # row p = local sk (plain), col j = 8*cq + rq (c-major sq)
    p = np.arange(128)
    cq, rq = np.arange(128) // 8, np.arange(128) % 8
    sq_loc = 16 * rq + cq                                  # [128]
    tri01 = np.where(p[:, None] <= sq_loc[None, :], 1.0, 0.0).astype(bf)
    return wqT, wkT, wvT, woT, csq1, csq2, csk1, csk2, tri01


def _build_program(dbg=False):
    import concourse.bass as bass
    import concourse.tile as tile
    from concourse import bacc, mybir

    f32 = mybir.dt.float32
    bf16 = mybir.dt.bfloat16
    EXP = mybir.ActivationFunctionType.Exp
    CPY = mybir.ActivationFunctionType.Copy

    nc = bacc.Bacc("TRN2", target_bir_lowering=False, debug=False,
                   num_devices=NC_N)

    xT = nc.dram_tensor("xT", [D, RPC], bf16, kind="ExternalInput").ap()
    wq = nc.dram_tensor("wqT", [D, D], bf16, kind="ExternalInput").ap()
    wk = nc.dram_tensor("wkT", [D, D], bf16, kind="ExternalInput").ap()
    wv = nc.dram_tensor("wvT", [D, D], bf16, kind="ExternalInput").ap()
    wo = nc.dram_tensor("woT", [D, D], bf16, kind="ExternalInput").ap()
    cs1d = nc.dram_tensor("cs1", [128, S], bf16, kind="ExternalInput").ap()
    cs2d = nc.dram_tensor("cs2", [128, S], bf16, kind="ExternalInput").ap()
    cs3d = nc.dram_tensor("cs3", [128, S], bf16, kind="ExternalInput").ap()
    cs4d = nc.dram_tensor("cs4", [128, S], bf16, kind="ExternalInput").ap()
    trid = nc.dram_tensor("tri", [128, 128], bf16, kind="ExternalInput").ap()
    y = nc.dram_tensor("y", [RPC, D], f32, kind="ExternalOutput").ap()
    vfd = nc.dram_tensor("vflat_scratch", [RPC, D], bf16).ap()

    with tile.TileContext(nc) as tc:
        with (
            tc.tile_pool(name="big", bufs=3) as big,        # wq/wk/wv -> outTA/outTB/y_sb
            tc.tile_pool(name="wo", bufs=1) as wop,
            tc.tile_pool(name="xp", bufs=1) as xp,
            tc.tile_pool(name="qk", bufs=2) as qkp,          # qraw, kraw (become rot in place)
            tc.tile_pool(name="cs", bufs=4) as csp,          # 4 rope tables
            tc.tile_pool(name="sw", bufs=2) as swp,          # rope swap scratch
            tc.tile_pool(name="vf", bufs=1) as vfp,
            tc.tile_pool(name="vsb", bufs=2) as vsbp,
            tc.tile_pool(name="mask", bufs=1) as maskp,
            tc.tile_pool(name="pp", bufs=18) as ppool,       # exp'd P chunks
            tc.tile_pool(name="ct", bufs=2) as ctp,          # concatT per (h, rt)
            tc.tile_pool(name="norm", bufs=2) as normp,      # dn row + bcast block
        ):
            # ---------- phase 0: loads ----------
            xsb = xp.tile([128, 8 * RPC], bf16, tag="x")           # [p, kt*512+r]
            xv = xsb[:].rearrange("p (kt r) -> p kt r", kt=8)
            xs = xT.rearrange("(kt p) r -> p kt r", p=128)
            for kt in range(8):
                nc.sync.dma_start(xv[:, kt, :], xs[:, kt, :])

            def load_w(pool, tag, src):
                t = pool.tile([128, 8 * D], bf16, tag=tag, name=f"w_{tag}")
                tv = t[:].rearrange("p (kt o) -> p kt o", kt=8)
                sv = src.rearrange("(kt p) o -> p kt o", p=128)
                for kt in range(8):
                    nc.sync.dma_start(tv[:, kt, :], sv[:, kt, :])
                return t

            wk_sb = load_w(big, "big", wk)
            wq_sb = load_w(big, "big", wq)
            cs3_sb = csp.tile([128, S], bf16, tag="cs")
            nc.sync.dma_start(cs3_sb[:], cs3d[:])
            cs4_sb = csp.tile([128, S], bf16, tag="cs")
            nc.sync.dma_start(cs4_sb[:], cs4d[:])
            cs1_sb = csp.tile([128, S], bf16, tag="cs")
            nc.sync.dma_start(cs1_sb[:], cs1d[:])
            cs2_sb = csp.tile([128, S], bf16, tag="cs")
            nc.sync.dma_start(cs2_sb[:], cs2d[:])
            wv_sb = load_w(big, "big", wv)
            wo_sb = load_w(wop, "wo", wo)
            tri_sb = maskp.tile([128, 128], bf16, tag="mask")
            nc.sync.dma_start(tri_sb[:], trid[:])

            qraw = qkp.tile([128, S], bf16, tag="qk")   # [Aev,Aod,Bev,Bod] x s
            kraw = qkp.tile([128, S], bf16, tag="qk")

            # vsb tiles early (gpsimd memset of the ones column is off-path)
            vsbs = []
            for h in range(HPC):
                vsb = vsbp.tile([128, 65 * NT_SK], bf16, tag="vsb")
                nc.gpsimd.memset(vsb[:], 1.0)   # ones col at 65T+64 survives
                vsbs.append(vsb)

            with tc.tile_pool(name="pssc", bufs=2, space="PSUM") as pssc:
                with tc.tile_pool(name="psproj", bufs=2, space="PSUM") as psp:
                    # ---------- phase 1a: K projection (copies on DVE — the
                    # strided dst is cheap there; ScalarE pays 3x for it) ----
                    kv_raw = kraw[:].rearrange("p (r c) -> p r c", c=16)
                    for ot in range(8):
                        pq = psp.tile([128, RPC], f32, tag="ps")
                        for kt in range(8):
                            nc.tensor.matmul(
                                pq[:],
                                wk_sb[:, kt * D + ot * 128: kt * D + ot * 128 + 128],
                                xsb[:, kt * RPC: (kt + 1) * RPC],
                                start=(kt == 0), stop=(kt == 7),
                            )
                        c0 = 2 * ot
                        nc.vector.tensor_copy(kv_raw[0:64, 0:256, c0],
                                              pq[0:64, 0:256])
                        nc.vector.tensor_copy(kv_raw[64:128, 0:256, c0],
                                              pq[0:64, 256:512])
                        nc.vector.tensor_copy(kv_raw[0:64, 0:256, c0 + 1],
                                              pq[64:128, 0:256])
                        nc.vector.tensor_copy(kv_raw[64:128, 0:256, c0 + 1],
                                              pq[64:128, 256:512])
                    # K rope in s-chunks of 1024 (chunk c covers sk tiles
                    # 8c..8c+7); scores for b=0 start after chunk 0
                    for kc in range(4):
                        lo, hi = 1024 * kc, 1024 * (kc + 1)
                        swk = swp.tile([128, 1024], bf16, tag="sw",
                                       name=f"swk{kc}")
                        nc.vector.tensor_copy(swk[0:32, 0:1024],
                                              kraw[32:64, lo:hi])
                        nc.vector.tensor_copy(swk[32:64, 0:1024],
                                              kraw[0:32, lo:hi])
                        nc.vector.tensor_copy(swk[64:96, 0:1024],
                                              kraw[96:128, lo:hi])
                        nc.vector.tensor_copy(swk[96:128, 0:1024],
                                              kraw[64:96, lo:hi])
                        nc.vector.tensor_mul(swk[:, 0:1024], swk[:, 0:1024],
                                             cs4_sb[:, lo:hi])
                        nc.vector.tensor_mul(kraw[:, lo:hi], kraw[:, lo:hi],
                                             cs3_sb[:, lo:hi])
                        nc.vector.tensor_add(kraw[:, lo:hi], kraw[:, lo:hi],
                                             swk[:, 0:1024])

                    # ---------- phase 1b: Q projection (copies on ScalarE:
                    # contiguous, and DVE is busy with K rope) ----------
                    for ot in range(8):
                        pq = psp.tile([128, RPC], f32, tag="ps")
                        for kt in range(8):
                            nc.tensor.matmul(
                                pq[:],
                                wq_sb[:, kt * D + ot * 128: kt * D + ot * 128 + 128],
                                xsb[:, kt * RPC: (kt + 1) * RPC],
                                start=(kt == 0), stop=(kt == 7),
                            )
                        c0 = 2 * ot
                        u0, u1 = 256 * c0, 256 * (c0 + 1)
                        nc.scalar.activation(qraw[0:64, u0:u0 + 256],
                                             pq[0:64, 0:256], CPY)
                        nc.scalar.activation(qraw[64:128, u0:u0 + 256],
                                             pq[0:64, 256:512], CPY)
                        nc.scalar.activation(qraw[0:64, u1:u1 + 256],
                                             pq[64:128, 0:256], CPY)
                        nc.scalar.activation(qraw[64:128, u1:u1 + 256],
                                             pq[64:128, 256:512], CPY)
                    # Q rope per q-block (strided r-slices of the c-major
                    # layout): block b's slice is ready right after block b-1,
                    # so the first scores start ~15us earlier and blocks 1-7
                    # rope under the attention stream
                    qrv = qraw[:].rearrange("p (c r) -> p c r", c=16)
                    c1v = cs1_sb[:].rearrange("p (c r) -> p c r", c=16)
                    c2v = cs2_sb[:].rearrange("p (c r) -> p c r", c=16)
                    for qb in range(8):
                        r0, r1 = 32 * qb, 32 * (qb + 1)
                        swq = swp.tile([128, 512], bf16, tag="swq",
                                       name=f"swq{qb}")
                        swv = swq[:].rearrange("p (c r) -> p c r", r=32)
                        nc.vector.tensor_copy(swv[0:32], qrv[32:64, :, r0:r1])
                        nc.vector.tensor_copy(swv[32:64], qrv[0:32, :, r0:r1])
                        nc.vector.tensor_copy(swv[64:96], qrv[96:128, :, r0:r1])
                        nc.vector.tensor_copy(swv[96:128], qrv[64:96, :, r0:r1])
                        nc.vector.tensor_mul(swv[:], swv[:], c2v[:, :, r0:r1])
                        nc.vector.tensor_mul(qrv[:, :, r0:r1], qrv[:, :, r0:r1],
                                             c1v[:, :, r0:r1])
                        nc.vector.tensor_add(qrv[:, :, r0:r1], qrv[:, :, r0:r1],
                                             swv[:])
                    qrot, krot = qraw, kraw

                    # ---------- phase 3: V projection + reshape ----------
                    # rt order (0,2,1,3): both heads' first halves reach vsb
                    # early.  Reshape goes through DRAM (strided gather).
                    vflat = vfp.tile([128, 4 * D], bf16, tag="vf")
                    vfd_v = vfd.rearrange("(rt p) o -> p rt o", p=128)
                    vld = vfd.rearrange("(h T a) (c d) -> h a c T d", h=2,
                                        T=NT_SK, a=8, c=16, d=DK)
                    for rt in (0, 2, 1, 3):
                        for ob in range(2):
                            pv = psp.tile([128, 512], f32, tag="ps")
                            for kt in range(8):
                                nc.tensor.matmul(
                                    pv[:],
                                    xsb[:, kt * RPC + rt * 128: kt * RPC + rt * 128 + 128],
                                    wv_sb[:, kt * D + ob * 512: kt * D + ob * 512 + 512],
                                    start=(kt == 0), stop=(kt == 7),
                                )
                            nc.scalar.activation(
                                vflat[:, rt * D + ob * 512: rt * D + ob * 512 + 512],
                                pv[:], CPY)
                        nc.sync.dma_start(vfd_v[:, rt, :],
                                          vflat[:].rearrange(
                                              "p (rt o) -> p rt o", rt=4)[:, rt, :])
                        # quirky map: vfd rows [256h+128*half, +128) = head h,
                        # tiles T in [16*half, 16*half+16)
                        h, half = divmod(rt, 2)
                        T0 = 16 * half
                        dstv = vsbs[h][:].rearrange("(a c) (T d) -> a c T d",
                                                    a=8, c=16, T=NT_SK, d=65)
                        for a in range(8):
                            nc.sync.dma_start(dstv[a, :, T0:T0 + 16, 0:DK],
                                              vld[h, a, :, T0:T0 + 16, :])

                # ---------- phase 4: attention ----------
                with tc.tile_pool(name="psout", bufs=4, space="PSUM") as psout:
                    outTs = []
                    for h in range(HPC):
                        outT = big.tile([64, S], bf16, tag="big")
                        outTs.append(outT)
                    y_sb = None
                    qv = [qrot[64 * h: 64 * h + 64, :].rearrange(
                        "p (c r) -> p c r", c=16) for h in range(HPC)]
                    triv = tri_sb[:].rearrange("p (c r) -> p c r", r=8)
                    yv = y.rearrange("(g p) o -> p g o", p=128)

                    def emit_oproj(h, rt, y_sb):
                        g = 2 * h + rt
                        ct = ctp.tile([128, 8 * 128], bf16, tag="ct")
                        for tp in range(8):
                            ue = 256 * (2 * tp) + 128 * rt
                            uo = 256 * (2 * tp + 1) + 128 * rt
                            nc.vector.tensor_copy(
                                ct[0:64, 128 * tp: 128 * tp + 128],
                                outTs[h][:, ue:ue + 128])
                            nc.vector.tensor_copy(
                                ct[64:128, 128 * tp: 128 * tp + 128],
                                outTs[h][:, uo:uo + 128])
                        for ob in range(2):
                            py = psout.tile([128, 512], f32, tag="out",
                                            name=f"py{g}_{ob}")
                            for tp in range(8):
                                nc.tensor.matmul(
                                    py[:],
                                    ct[:, 128 * tp: 128 * tp + 128],
                                    wo_sb[:, tp * D + ob * 512: tp * D + ob * 512 + 512],
                                    start=(tp == 0), stop=(tp == 7),
                                )
                            nc.vector.tensor_copy(
                                y_sb[:, g * D + ob * 512: g * D + ob * 512 + 512],
                                py[:])
                        nc.sync.dma_start(yv[:, g, :], y_sb[:, g * D:(g + 1) * D])

                    for b in range(8):
                        nt = 4 * (b + 1)
                        outp = [psout.tile([65, 512], f32, tag="out",
                                           name=f"outp{b}_{hh}")
                                for hh in range(HPC)]
                        opv = [outp[hh][:].rearrange("p (c r) -> p c r", r=32)
                               for hh in range(HPC)]
                        slots = [(t, h) for t in range(nt) for h in range(HPC)]
                        chunks = [slots[i:i + 2] for i in range(0, len(slots), 2)]

                        def emit_pv(chunk, pch):
                            for j, (t, h) in enumerate(chunk):
                                rmin = 8 * max(0, t - 4 * b)
                                pcv = pch[:, 512 * j: 512 * (j + 1)].rearrange(
                                    "p (c r) -> p c r", r=32)
                                nc.tensor.matmul(
                                    opv[h][:, :, rmin:32],
                                    vsbs[h][:, 65 * t: 65 * t + 65],
                                    pcv[:, :, rmin:32],
                                    start=(t == 0), stop=(t == nt - 1),
                                )

                        pend = None   # PV emission delayed one chunk: the PE
                        # stream reads [QK_i+1, PV_i, ...] so the in-order PE
                        # queue never idles waiting for exp_i
                        for chunk in chunks:
                            ps = pssc.tile([128, 1024], f32, tag="sc")
                            pch = ppool.tile([128, 1024], bf16, tag="pp")
                            # slot layout: uniform c-major-32 groups; valid
                            # r-range [rmin, 32) per c-group, garbage at
                            # [0, rmin) (never read downstream)
                            for j, (t, h) in enumerate(chunk):
                                rmin = 8 * max(0, t - 4 * b)
                                psv = ps[:, 512 * j: 512 * (j + 1)].rearrange(
                                    "p (c r) -> p c r", r=32)
                                nc.tensor.matmul(
                                    psv[:, :, rmin:32],
                                    krot[64 * h: 64 * h + 64,
                                         128 * t: 128 * t + 128],
                                    qv[h][:, :, 32 * b + rmin: 32 * (b + 1)],
                                    start=True, stop=True,
                                )
                            Wtot = 512 * len(chunk)
                            nc.scalar.activation(pch[:, 0:Wtot], ps[:, 0:Wtot],
                                                 EXP, scale=0.125)
                            # diagonal tiles: zero the upper triangle of the
                            # exp'd P (multiplicative 0/1 mask, SBUF bf16)
                            for j, (t, h) in enumerate(chunk):
                                if t >= 4 * b:
                                    rmin = 8 * (t - 4 * b)
                                    pm = pch[:, 512 * j: 512 * (j + 1)].rearrange(
                                        "p (c r) -> p c r", r=32)[:, :, rmin:rmin + 8]
                                    nc.vector.tensor_mul(pm, pm, triv)
                            if pend is not None:
                                emit_pv(*pend)
                            pend = (chunk, pch)
                        emit_pv(*pend)
                        # normalize block b: row 64 of outp holds the softmax
                        # denominators (ones column of [V|1] stationary)
                        for h in range(HPC):
                            nrm = normp.tile([128, 1024], f32, tag="norm",
                                             name=f"nrm{b}_{h}")
                            nc.vector.tensor_copy(nrm[0:1, 512:1024],
                                                  outp[h][64:65, :])
                            nc.vector.reciprocal_approx_fast(
                                out=nrm[0:1, 0:512], in_=nrm[0:1, 512:1024])
                            nc.vector.stream_shuffle(nrm[64:96, 0:512],
                                                     nrm[0:32, 0:512], [0] * 32)
                            nc.vector.stream_shuffle(nrm[96:128, 0:512],
                                                     nrm[0:32, 0:512], [0] * 32)
                            osl = outTs[h][:].rearrange(
                                "p (c r) -> p c r", r=256)[:, :, 32 * b: 32 * (b + 1)]
                            nc.vector.tensor_mul(osl, outp[h][0:64, :],
                                                 nrm[64:128, 0:512])
                            # ---------- phase 5: output projection pieces ----
                            # (h, rt) emitted as soon as its 4 q-blocks are
                            # normalized; at b==7 interleave with the other
                            # head's normalize to shorten the tail
                            if b == 3 and h == HPC - 1:
                                y_sb = big.tile([128, 4 * D], f32, tag="big")
                                for hh in range(HPC):
                                    emit_oproj(hh, 0, y_sb)
                            elif b == 7:
                                emit_oproj(h, 1, y_sb)

    nc.compile()
    return nc


def kernel(**inputs):
    x = np.asarray(inputs["x"], dtype=np.float32)     # [1, 4096, 1024]
    Wq = np.asarray(inputs["Wq"], dtype=np.float32)
    Wk = np.asarray(inputs["Wk"], dtype=np.float32)
    Wv = np.asarray(inputs["Wv"], dtype=np.float32)
    Wo = np.asarray(inputs["Wo"], dtype=np.float32)
    # biases are structurally zero in this problem; fold anyway if nonzero
    for bn in ("bq", "bk", "bv", "bo"):
        bv_ = np.asarray(inputs.get(bn, 0.0))
        assert np.all(bv_ == 0.0), f"{bn} nonzero: unsupported"

    from concourse.bass_utils import run_bass_kernel_spmd

    if "nc" not in _CACHE:
        _CACHE["nc"] = _build_program()
    nc = _CACHE["nc"]

    bf = ml_dtypes.bfloat16
    wqT, wkT, wvT, woT, csq1, csq2, csk1, csk2, tri01 = _host_arrays(
        Wq, Wk, Wv, Wo)
    shared = {"wqT": wqT, "wkT": wkT, "wvT": wvT, "woT": woT,
              "cs1": csq1, "cs2": csq2, "cs3": csk1, "cs4": csk2,
              "tri": tri01}
    xf = x.reshape(S, D)
    in_maps = []
    for i in range(NC_N):
        xTi = np.ascontiguousarray(xf[i * RPC:(i + 1) * RPC, :].T).astype(bf)
        in_maps.append(dict(shared, xT=xTi))

    trace = bool(int(os.environ.get("BASS_KERNEL_TRACE", "0")))
    res = run_bass_kernel_spmd(nc, in_maps, core_ids=list(range(NC_N)),
                               trace=trace)
    _CACHE["last_res"] = res
    if trace and res.exec_time_ns is not None:
        print(f"HW exec time: {res.exec_time_ns} ns")
        _CACHE["exec_time_ns"] = res.exec_time_ns
        _CACHE["trace"] = res.instructions_and_trace
    out = np.concatenate([res.results[i]["y"] for i in range(NC_N)], axis=0)
    return out.reshape(1, S, D).astype(np.float32)
